# revision 7
# baseline (speedup 1.0000x reference)
"""Trainium2 Bass kernel for nn_AttnMixer (2D-local sparse attention).

Strategy: data-parallel over batch N=32 across 8 cores (4 batches/core).
Positions are permuted host-side to w-major order (i = w*16 + h), which
makes the local window [qpos-80, qpos+208) only 288 wide (vs 512 in
h-major).  Per (q-tile of 128, head):
  scores = qT.T @ kT_window (one K=64 MM, N=288) + rank-24 additive mask
  MM (-30 off-window), exp+row-sum fused on ACT (accum_out -> z),
  rz = recip(z) batched per tile, probs = e * rz (DVE), PE-transpose
  3 chunks, AV matmuls accumulate o^T[d, q], proj per s-tile.
v is computed by matmul directly into the two shifted chunk grids the
windows need (aligned + 48-offset), so no SBUF shift DMAs.
b_proj is added host-side (it is zeros in this problem anyway).
"""

import numpy as np

N_CORES = 8
N, S, C = 32, 1024, 512
NB = N // N_CORES
GH, GW = 16, 64          # original grid (h-major); we permute to w-major
NH, HD = 8, 64
NT = S // 128            # 8 q-tiles
W_WIN = 288              # window width in w-major order
BIGNEG = -30.0

# window start per q-tile t (w-major coords), all multiples of 16
_STARTS = [0, 48, 176, 304, 432, 560, 688, 816]
_UVAR = [0, 1, 1, 1, 1, 1, 1, 2]   # u-table variant per tile

_CACHE = {}


def _host_consts():
    # ind [24,128]: rows 0-7 indicator(q//16 == d), rows 8-23 indicator(q%16 == d2)
    q = np.arange(128)
    ind = np.zeros((24, 128), dtype=np.float16)
    for d in range(8):
        ind[d] = (q // 16 == d)
    for d2 in range(16):
        ind[8 + d2] = (q % 16 == d2)
    # u_tab [3,24,288]: additive mask tables (0 allowed / BIGNEG disallowed)
    j = np.arange(W_WIN)
    jw, jh = j // 16, j % 16
    u = np.zeros((3, 24, W_WIN), dtype=np.float32)
    for d in range(8):
        u[0, d] = BIGNEG * (np.abs(d - jw) > 5)            # t=0 (start 0)
        u[1, d] = BIGNEG * (np.abs(d + 5 - jw) > 5)        # interior
        u[2, d] = BIGNEG * ((np.abs(d + 5 - jw) > 5) | (jw >= 13))  # t=7
    for d2 in range(16):
        u[:, 8 + d2] = BIGNEG * (np.abs(d2 - jh) > 3)[None, :]
    return ind, u.astype(np.float16)


def _build_bass():
    import concourse.tile as tile
    from concourse import bacc, mybir

    f32, f16 = mybir.dt.float32, mybir.dt.float16

    nc = bacc.Bacc("TRN2", target_bir_lowering=False)
    xt = nc.dram_tensor("xt", [NB, 512, 1024], f16, kind="ExternalInput")
    wqk = nc.dram_tensor("wqk", [512, 1024], f16, kind="ExternalInput")
    wv = nc.dram_tensor("wv", [512, 512], f16, kind="ExternalInput")
    wp = nc.dram_tensor("wp", [512, 512], f16, kind="ExternalInput")
    identd = nc.dram_tensor("identd", [128, 128], f16, kind="ExternalInput")
    indd = nc.dram_tensor("indd", [24, 128], f16, kind="ExternalInput")
    utabd = nc.dram_tensor("utabd", [3, 24, W_WIN], f16, kind="ExternalInput")
    y = nc.dram_tensor("y", [NB, 1024, 512], f16, kind="ExternalOutput")

    # v chunk grids:
    #  aligned (t=0): chunks at rows [0,128), [128,256), [256,288) (M=32)
    #  48-offset (t=1..7): chunk j at rows [128j+48, 128j+176), j=0..6 full,
    #  j=7 rows [944,1024) (M=80, partitions 80..127 zeroed)
    with tile.TileContext(nc) as tc:
        with tc.tile_pool(name="const", bufs=1) as const, \
             tc.tile_pool(name="xtp", bufs=2) as xtp, \
             tc.tile_pool(name="qkp", bufs=2) as qkp, \
             tc.tile_pool(name="vp", bufs=2) as vp, \
             tc.tile_pool(name="otp", bufs=2) as otp, \
             tc.tile_pool(name="work", bufs=2) as work, \
             tc.tile_pool(name="yout", bufs=3) as yout, \
             tc.tile_pool(name="psA", bufs=2, space="PSUM") as psA, \
             tc.tile_pool(name="psS", bufs=2, space="PSUM") as psS, \
             tc.tile_pool(name="psT", bufs=2, space="PSUM") as psT, \
             tc.tile_pool(name="psO", bufs=2, space="PSUM") as psO:

            # ---- constants ----
            wqk_sb = const.tile([128, 4, 1024], f16)
            nc.gpsimd.dma_start(out=wqk_sb, in_=wqk.rearrange("(k p) m -> p k m", p=128))
            wv_sb = const.tile([128, 4, 512], f16)
            nc.gpsimd.dma_start(out=wv_sb, in_=wv.rearrange("(k p) m -> p k m", p=128))
            wp_sb = const.tile([128, 4, 512], f16)
            nc.gpsimd.dma_start(out=wp_sb, in_=wp.rearrange("(k p) m -> p k m", p=128))
            ident_sb = const.tile([128, 128], f16)
            nc.gpsimd.dma_start(out=ident_sb, in_=identd[:, :])
            ind_sb = const.tile([24, 128], f16)
            nc.gpsimd.dma_start(out=ind_sb, in_=indd[:, :])
            u_sb = const.tile([24, 3, W_WIN], f16)
            nc.gpsimd.dma_start(out=u_sb, in_=utabd.rearrange("v p m -> p v m"))

            for n in range(NB):
                # ---- phase A: qkv projections ----
                xt_sb = xtp.tile([128, 4, 1024], f16)
                nc.gpsimd.dma_start(
                    out=xt_sb, in_=xt[n].rearrange("(k p) s -> p k s", p=128)
                )

                # qkT [128, 8 chunks, 1152] fp16 (cols 1024:1152 zero pad for t=7)
                qkT = qkp.tile([128, 8, 1152], f16)
                nc.gpsimd.memset(qkT[:, :, 1024:1152], 0.0)
                for m in range(8):
                    for sh in range(2):
                        ps = psA.tile([128, 512], f32, tag="A")
                        for k in range(4):
                            nc.tensor.matmul(
                                ps,
                                wqk_sb[:, k, m * 128:(m + 1) * 128],
                                xt_sb[:, k, sh * 512:(sh + 1) * 512],
                                start=(k == 0),
                                stop=(k == 3),
                            )
                        nc.any.tensor_copy(qkT[:, m, sh * 512:(sh + 1) * 512], ps)

                # v tiles
                val = vp.tile([128, 3, 512], f16, tag="val")
                v48 = vp.tile([128, 8, 512], f16, tag="v48")
                nc.gpsimd.memset(v48[64:128, 7, :], 0.0)
                vjobs = []
                for jj in range(2):
                    vjobs.append((val, jj, jj * 128, 128))
                vjobs.append((val, 2, 256, 32))
                for jj in range(7):
                    vjobs.append((v48, jj, 128 * jj + 48, 128))
                vjobs.append((v48, 7, 944, 80))
                for (vdst, slot, row0, m) in vjobs:
                    ps = psA.tile([128, 512], f32, tag="A")
                    for k in range(4):
                        nc.tensor.matmul(
                            ps[0:m, :],
                            xt_sb[:, k, row0:row0 + m],
                            wv_sb[:, k, :],
                            start=(k == 0),
                            stop=(k == 3),
                        )
                    nc.any.tensor_copy(vdst[0:m, slot, :], ps[0:m, :])

                # ---- phase B: local attention ----
                outT = otp.tile([128, 4, 1024], f16)
                for t in range(NT):
                    stk = _STARTS[t]
                    uv = _UVAR[t]
                    nch = 2 if t == 7 else 3
                    z_t = work.tile([128, 8], f32, tag="z")
                    e_tiles = []
                    for h in range(NH):
                        hp, p0 = h // 2, 64 * (h % 2)
                        ps_s = psS.tile([128, W_WIN], f32, tag="S")
                        nc.tensor.matmul(
                            ps_s,
                            qkT[p0:p0 + 64, hp, t * 128:(t + 1) * 128],
                            qkT[p0:p0 + 64, 4 + hp, stk:stk + W_WIN],
                            start=True,
                            stop=False,
                        )
                        nc.tensor.matmul(
                            ps_s,
                            ind_sb[:, :],
                            u_sb[:, uv, :],
                            start=False,
                            stop=True,
                        )
                        e = work.tile([128, W_WIN], f16, tag=f"e{h}")
                        nc.scalar.activation(
                            e, ps_s, mybir.ActivationFunctionType.Exp,
                            accum_out=z_t[:, h:h + 1],
                        )
                        e_tiles.append(e)
                    rz_t = work.tile([128, 8], f32, tag="rz")
                    nc.vector.reciprocal(rz_t, z_t)

                    ps_o = psO.tile([128, 4, 128], f32, tag="O")
                    for h in range(NH):
                        hp, p0 = h // 2, 64 * (h % 2)
                        # diag(rz_h): transpose-matmuls below fold the softmax
                        # normalization and stay HAM-visible (real matmuls)
                        diag = work.tile([128, 128], f16, tag=f"dg{h % 4}")
                        nc.vector.tensor_scalar_mul(
                            diag, in0=ident_sb, scalar1=rz_t[:, h:h + 1]
                        )
                        ps_t = psT.tile([128, 384], f32, tag="T")
                        for j in range(nch):
                            m = 32 if j == 2 else 128
                            nc.tensor.matmul(
                                ps_t[0:m, j * 128:(j + 1) * 128],
                                e_tiles[h][:, j * 128:j * 128 + m],
                                diag,
                                start=True,
                                stop=True,
                            )
                        aT = work.tile([128, 384], f16, tag=f"aT{h % 4}")
                        nc.any.tensor_copy(
                            aT[:, 0:nch * 128], ps_t[:, 0:nch * 128]
                        )
                        if t == 0:
                            vchunks = [(val, 0, 128), (val, 1, 128), (val, 2, 32)]
                        elif t < 7:
                            vchunks = [(v48, t - 1, 128), (v48, t, 128), (v48, t + 1, 32)]
                        else:
                            vchunks = [(v48, 6, 128), (v48, 7, 128)]
                        for j, (vsrc, slot, kk) in enumerate(vchunks):
                            nc.tensor.matmul(
                                ps_o[p0:p0 + 64, hp, :],
                                vsrc[0:kk, slot, h * 64:(h + 1) * 64],
                                aT[0:kk, j * 128:(j + 1) * 128],
                                start=(j == 0),
                                stop=(j == len(vchunks) - 1),
                                skip_group_check=True,
                            )
                    nc.any.tensor_copy(outT[:, :, t * 128:(t + 1) * 128], ps_o)

                # ---- phase C: output projection ----
                for st in range(8):
                    ps_y = psA.tile([128, 512], f32, tag="A")
                    for k in range(4):
                        nc.tensor.matmul(
                            ps_y,
                            outT[:, k, st * 128:(st + 1) * 128],
                            wp_sb[:, k, :],
                            start=(k == 0),
                            stop=(k == 3),
                        )
                    y_sb = yout.tile([128, 512], f16, tag="y")
                    nc.any.tensor_copy(y_sb, ps_y)
                    nc.gpsimd.dma_start(
                        out=y[n, st * 128:(st + 1) * 128, :], in_=y_sb
                    )

    nc.finalize()
    return nc


def _install_prof_shim():
    import sys
    import types
    if "antenv.axon_hooks" in sys.modules:
        return
    try:
        from trn_agent_boot.trn_boot import _ntff_profile_via_ctypes
        hook = _ntff_profile_via_ctypes("/opt/axon/libaxon_pjrt.so")
    except Exception:
        return
    mod = types.ModuleType("antenv.axon_hooks")
    mod.get_axon_ntff_profile_hook = lambda: hook
    mod.set_axon_ntff_profile_hook = lambda h: None
    sys.modules["antenv.axon_hooks"] = mod


def _run(inputs, trace=False):
    from concourse.bass_utils import run_bass_kernel_spmd

    if trace:
        _install_prof_shim()
    if "nc" not in _CACHE:
        _CACHE["nc"] = _build_bass()
    nc = _CACHE["nc"]

    x = np.ascontiguousarray(inputs["x"], dtype=np.float32)
    w_qkv = np.asarray(inputs["W_qkv"], dtype=np.float32)
    w_proj = np.ascontiguousarray(inputs["W_proj"], dtype=np.float32)
    b_proj = np.asarray(inputs["b_proj"], dtype=np.float32)

    wqk = w_qkv[:, : 2 * C].copy()
    wqk[:, :C] *= HD ** -0.5
    wqk = wqk.astype(np.float16)
    wvf = np.ascontiguousarray(w_qkv[:, 2 * C:]).astype(np.float16)
    wpf = w_proj.astype(np.float16)
    ident = np.eye(128, dtype=np.float16)
    ind, utab = _host_consts()

    # w-major permutation: new index i = w*16 + h  ->  old index p = h*64 + w
    i = np.arange(S)
    perm = (i % GH) * GW + (i // GH)      # old position for new index i
    inv = np.empty(S, dtype=np.int64)
    inv[perm] = i

    xtn = x.transpose(0, 2, 1)[:, :, perm].astype(np.float16)  # [N, C, S]
    in_maps = []
    for c in range(N_CORES):
        in_maps.append({
            "xt": np.ascontiguousarray(xtn[c * NB:(c + 1) * NB]),
            "wqk": wqk, "wv": wvf, "wp": wpf,
            "identd": ident, "indd": ind, "utabd": utab,
        })

    res = run_bass_kernel_spmd(
        nc, in_maps, core_ids=list(range(N_CORES)), trace=trace
    )
    out = np.concatenate([res.results[c]["y"] for c in range(N_CORES)], axis=0)
    out = out[:, inv, :].astype(np.float32) + b_proj[None, None, :]
    return out, res


def kernel(**inputs):
    out, _ = _run(inputs, trace=False)
    return out


# revision 9
# speedup vs baseline: 1.7401x; 1.7401x over previous
"""Trainium2 Bass kernel for nn_AttnMixer (2D-local sparse attention).

Strategy: data-parallel over batch N=32 across 8 cores (4 batches/core).
Per core, per batch:
  A) qkT[2C, S] = Wqk.T @ xT and v[S, C], fp16 operands / fp32 PSUM
     (q pre-scaled by hd^-0.5 host-side).
  B) per (q-tile of 128, head-pair): scores = qT.T @ kT_window (K=64
     row-packed pairs into one [128,1024] PSUM), exp on ACT -> fp16,
     binary-mask multiply + fused row-sum on DVE, normalize, PE-transpose
     probs, AV matmuls col-packed per head pair accumulating
     o^T[d, qi] -> out^T[C, S].
  C) proj: y = outT.T @ Wp, bias fused into the PSUM->SBUF copy, DMA out.
All shapes hardcoded; host side only reshapes/transposes numpy inputs.
"""

import numpy as np

N_CORES = 8
N, S, C = 32, 1024, 512
NB = N // N_CORES
GH, GW = 16, 64
HK, WK = 7, 11
NH, HD = 8, 64
NT = S // 128  # 8 q-tiles per (n, h)


def _win_start(t):
    return min(max(128 * t - 192, 0), 512)


def _build_masks():
    mk = np.zeros((NT, 128, 512), dtype=np.float32)
    for t in range(NT):
        stk = _win_start(t)
        qs = t * 128 + np.arange(128)
        ks = stk + np.arange(512)
        qh, qw = qs // GW, qs % GW
        kh, kw = ks // GW, ks % GW
        ok = (np.abs(qh[:, None] - kh[None, :]) <= HK // 2) & (
            np.abs(qw[:, None] - kw[None, :]) <= WK // 2
        )
        mk[t] = ok
    return mk.astype(np.float16)


_CACHE = {}


def _build_bass():
    import concourse.tile as tile
    from concourse import bacc, mybir

    f32, f16 = mybir.dt.float32, mybir.dt.float16

    nc = bacc.Bacc("TRN2", target_bir_lowering=False)
    xt = nc.dram_tensor("xt", [NB, 512, 1024], f16, kind="ExternalInput")
    wqk = nc.dram_tensor("wqk", [512, 1024], f16, kind="ExternalInput")
    wv = nc.dram_tensor("wv", [512, 512], f16, kind="ExternalInput")
    wp = nc.dram_tensor("wp", [512, 512], f16, kind="ExternalInput")
    bfull = nc.dram_tensor("bfull", [128, 512], f32, kind="ExternalInput")
    identd = nc.dram_tensor("identd", [128, 128], f16, kind="ExternalInput")
    maskd = nc.dram_tensor("maskd", [NT, 128, 512], f16, kind="ExternalInput")
    y = nc.dram_tensor("y", [NB, 1024, 512], f32, kind="ExternalOutput")

    with tile.TileContext(nc) as tc:
        with tc.tile_pool(name="const", bufs=1) as const, \
             tc.tile_pool(name="xtp", bufs=2) as xtp, \
             tc.tile_pool(name="qkp", bufs=2) as qkp, \
             tc.tile_pool(name="vp", bufs=2) as vp, \
             tc.tile_pool(name="otp", bufs=2) as otp, \
             tc.tile_pool(name="work", bufs=4) as work, \
             tc.tile_pool(name="yout", bufs=3) as yout, \
             tc.tile_pool(name="psA", bufs=2, space="PSUM") as psA, \
             tc.tile_pool(name="psT", bufs=2, space="PSUM") as psT:

            # ---- constants ----
            wqk_sb = const.tile([128, 4, 1024], f16)
            nc.gpsimd.dma_start(out=wqk_sb, in_=wqk.rearrange("(k p) m -> p k m", p=128))
            wv_sb = const.tile([128, 4, 512], f16)
            nc.gpsimd.dma_start(out=wv_sb, in_=wv.rearrange("(k p) m -> p k m", p=128))
            wp_sb = const.tile([128, 4, 512], f16)
            nc.gpsimd.dma_start(out=wp_sb, in_=wp.rearrange("(k p) m -> p k m", p=128))
            bias_sb = const.tile([128, 512], f32)
            nc.gpsimd.dma_start(out=bias_sb, in_=bfull[:, :])
            ident_sb = const.tile([128, 128], f16)
            nc.gpsimd.dma_start(out=ident_sb, in_=identd[:, :])
            mask_sb = const.tile([128, NT, 512], f16)
            nc.gpsimd.dma_start(out=mask_sb, in_=maskd.rearrange("t p m -> p t m"))

            for n in range(NB):
                # ---- phase A: projections ----
                xt_sb = xtp.tile([128, 4, 1024], f16)
                nc.gpsimd.dma_start(
                    out=xt_sb, in_=xt[n].rearrange("(k p) s -> p k s", p=128)
                )

                qkT = qkp.tile([128, 8, 1024], f16)
                for m in range(8):
                    ps = psA.tile([128, 1024], f32, tag="A")
                    for k in range(4):
                        for sh in range(2):
                            nc.tensor.matmul(
                                ps[:, sh * 512:(sh + 1) * 512],
                                wqk_sb[:, k, m * 128:(m + 1) * 128],
                                xt_sb[:, k, sh * 512:(sh + 1) * 512],
                                start=(k == 0),
                                stop=(k == 3),
                            )
                    nc.any.tensor_copy(qkT[:, m, :], ps)

                v_ev = vp.tile([128, 8, 512], f16, tag="ve")
                v_od = vp.tile([128, 7, 512], f16, tag="vo")
                for st in range(8):
                    ps = psA.tile([128, 512], f32, tag="A")
                    for k in range(4):
                        nc.tensor.matmul(
                            ps,
                            xt_sb[:, k, st * 128:(st + 1) * 128],
                            wv_sb[:, k, :],
                            start=(k == 0),
                            stop=(k == 3),
                        )
                    nc.any.tensor_copy(v_ev[:, st, :], ps)
                for m in range(7):
                    nc.gpsimd.dma_start(out=v_od[0:64, m, :], in_=v_ev[64:128, m, :])
                    nc.gpsimd.dma_start(out=v_od[64:128, m, :], in_=v_ev[0:64, m + 1, :])

                # ---- phase B: local attention ----
                outT = otp.tile([128, 4, 1024], f16)
                for t in range(NT):
                    stk = _win_start(t)
                    if stk % 128 == 0:
                        vsrc, vbase = v_ev, stk // 128
                    else:
                        vsrc, vbase = v_od, (stk - 64) // 128
                    for hp in range(4):
                        # paired scores: head 2hp on array rows 0:64 ->
                        # bank cols 0:512, head 2hp+1 on rows 64:128 ->
                        # cols 512:1024 (different PSUM banks, concurrent)
                        ps_s = psA.tile([128, 1024], f32, tag="A")
                        for ho in range(2):
                            p0 = ho * 64
                            nc.tensor.matmul(
                                ps_s[:, ho * 512:(ho + 1) * 512],
                                qkT[p0:p0 + 64, hp, t * 128:(t + 1) * 128],
                                qkT[p0:p0 + 64, 4 + hp, stk:stk + 512],
                                start=True,
                                stop=True,
                            )
                        e = work.tile([128, 1024], f16, tag="e")
                        nc.scalar.activation(e, ps_s, mybir.ActivationFunctionType.Exp)

                        ps_o = psT.tile([128, 128], f32, tag="oT")
                        for ho in range(2):
                            h = 2 * hp + ho
                            eM = work.tile([128, 512], f16, tag=f"eM{ho}")
                            z = work.tile([128, 1], f32, tag=f"z{ho}")
                            nc.vector.scalar_tensor_tensor(
                                out=eM, in0=e[:, ho * 512:(ho + 1) * 512],
                                scalar=1.0, in1=mask_sb[:, t, :],
                                op0=mybir.AluOpType.bypass,
                                op1=mybir.AluOpType.mult,
                                accum_out=z,
                            )
                            rz = work.tile([128, 1], f32, tag=f"rz{ho}")
                            nc.vector.reciprocal(rz, z)
                            probs = work.tile([128, 512], f16, tag=f"probs{ho}")
                            nc.vector.tensor_scalar_mul(probs, in0=eM, scalar1=rz)
                            ps_t = psT.tile([128, 512], f16, tag="aT")
                            for j in range(4):
                                nc.tensor.transpose(
                                    ps_t[:, j * 128:(j + 1) * 128],
                                    probs[:, j * 128:(j + 1) * 128],
                                    ident_sb,
                                )
                            aT = work.tile([128, 512], f16, tag=f"aTs{ho}")
                            nc.any.tensor_copy(aT, ps_t)
                            for j in range(4):
                                nc.tensor.matmul(
                                    ps_o[ho * 64:(ho + 1) * 64, :],
                                    vsrc[:, vbase + j, h * 64:(h + 1) * 64],
                                    aT[:, j * 128:(j + 1) * 128],
                                    start=(j == 0),
                                    stop=(j == 3),
                                    skip_group_check=True,
                                )
                        nc.any.tensor_copy(outT[:, hp, t * 128:(t + 1) * 128], ps_o)

                # ---- phase C: output projection ----
                for st in range(8):
                    ps_y = psA.tile([128, 512], f32, tag="A")
                    for k in range(4):
                        nc.tensor.matmul(
                            ps_y,
                            outT[:, k, st * 128:(st + 1) * 128],
                            wp_sb[:, k, :],
                            start=(k == 0),
                            stop=(k == 3),
                        )
                    y_sb = yout.tile([128, 512], f32, tag="y")
                    nc.vector.scalar_tensor_tensor(
                        out=y_sb, in0=ps_y, scalar=1.0, in1=bias_sb,
                        op0=mybir.AluOpType.bypass, op1=mybir.AluOpType.add,
                    )
                    nc.gpsimd.dma_start(
                        out=y[n, st * 128:(st + 1) * 128, :], in_=y_sb
                    )

    nc.finalize()
    return nc


def _install_prof_shim():
    import sys
    import types
    if "antenv.axon_hooks" in sys.modules:
        return
    try:
        from trn_agent_boot.trn_boot import _ntff_profile_via_ctypes
        hook = _ntff_profile_via_ctypes("/opt/axon/libaxon_pjrt.so")
    except Exception:
        return
    mod = types.ModuleType("antenv.axon_hooks")
    mod.get_axon_ntff_profile_hook = lambda: hook
    mod.set_axon_ntff_profile_hook = lambda h: None
    sys.modules["antenv.axon_hooks"] = mod


def _run(inputs, trace=False):
    from concourse.bass_utils import run_bass_kernel_spmd

    if trace:
        _install_prof_shim()
    if "nc" not in _CACHE:
        _CACHE["nc"] = _build_bass()
    nc = _CACHE["nc"]

    x = np.ascontiguousarray(inputs["x"], dtype=np.float32)
    w_qkv = np.asarray(inputs["W_qkv"], dtype=np.float32)
    w_proj = np.ascontiguousarray(inputs["W_proj"], dtype=np.float32)
    b_proj = np.asarray(inputs["b_proj"], dtype=np.float32)

    wqk = w_qkv[:, : 2 * C].copy()
    wqk[:, :C] *= HD ** -0.5
    wqk = wqk.astype(np.float16)
    wv = np.ascontiguousarray(w_qkv[:, 2 * C:]).astype(np.float16)
    wpf = w_proj.astype(np.float16)
    bfull = np.tile(b_proj[None, :], (128, 1)).astype(np.float32)
    ident = np.eye(128, dtype=np.float16)
    masks = _build_masks()

    xtn = x.transpose(0, 2, 1).astype(np.float16)  # [N, C, S]
    in_maps = []
    for c in range(N_CORES):
        in_maps.append({
            "xt": np.ascontiguousarray(xtn[c * NB:(c + 1) * NB]),
            "wqk": wqk, "wv": wv, "wp": wpf,
            "bfull": bfull, "identd": ident, "maskd": masks,
        })

    res = run_bass_kernel_spmd(
        nc, in_maps, core_ids=list(range(N_CORES)), trace=trace
    )
    out = np.concatenate([res.results[c]["y"] for c in range(N_CORES)], axis=0)
    return out.astype(np.float32), res


def kernel(**inputs):
    out, _ = _run(inputs, trace=False)
    return out



# revision 10
# speedup vs baseline: 1.7464x; 1.0036x over previous
"""Trainium2 Bass kernel for nn_AttnMixer (2D-local sparse attention).

Strategy: data-parallel over batch N=32 across 8 cores (4 batches/core).
Per core, per batch:
  A) qkT[2C, S] = Wqk.T @ xT and v[S, C], fp16 operands / fp32 PSUM
     (q pre-scaled by hd^-0.5 host-side).
  B) per (q-tile of 128, head-pair): scores = qT.T @ kT_window (K=64
     row-packed pairs into one [128,1024] PSUM), exp on ACT -> fp16,
     binary-mask multiply + fused row-sum on DVE, normalize, PE-transpose
     probs, AV matmuls col-packed per head pair accumulating
     o^T[d, qi] -> out^T[C, S].
  C) proj: y = outT.T @ Wp, bias fused into the PSUM->SBUF copy, DMA out.
All shapes hardcoded; host side only reshapes/transposes numpy inputs.
"""

import numpy as np

N_CORES = 8
N, S, C = 32, 1024, 512
NB = N // N_CORES
GH, GW = 16, 64
HK, WK = 7, 11
NH, HD = 8, 64
NT = S // 128  # 8 q-tiles per (n, h)


def _win_start(t):
    return min(max(128 * t - 192, 0), 512)


def _build_masks():
    mk = np.zeros((NT, 128, 512), dtype=np.float32)
    for t in range(NT):
        stk = _win_start(t)
        qs = t * 128 + np.arange(128)
        ks = stk + np.arange(512)
        qh, qw = qs // GW, qs % GW
        kh, kw = ks // GW, ks % GW
        ok = (np.abs(qh[:, None] - kh[None, :]) <= HK // 2) & (
            np.abs(qw[:, None] - kw[None, :]) <= WK // 2
        )
        mk[t] = ok
    return mk.astype(np.float16)


_CACHE = {}


def _build_bass():
    import concourse.tile as tile
    from concourse import bacc, mybir

    f32, f16 = mybir.dt.float32, mybir.dt.float16

    nc = bacc.Bacc("TRN2", target_bir_lowering=False)
    xt = nc.dram_tensor("xt", [NB, 512, 1024], f16, kind="ExternalInput")
    wqk = nc.dram_tensor("wqk", [512, 1024], f16, kind="ExternalInput")
    wv = nc.dram_tensor("wv", [512, 512], f16, kind="ExternalInput")
    wp = nc.dram_tensor("wp", [512, 512], f16, kind="ExternalInput")
    bfull = nc.dram_tensor("bfull", [128, 512], f32, kind="ExternalInput")
    identd = nc.dram_tensor("identd", [128, 128], f16, kind="ExternalInput")
    maskd = nc.dram_tensor("maskd", [NT, 128, 512], f16, kind="ExternalInput")
    y = nc.dram_tensor("y", [NB, 1024, 512], f32, kind="ExternalOutput")

    with tile.TileContext(nc) as tc:
        with tc.tile_pool(name="const", bufs=1) as const, \
             tc.tile_pool(name="xtp", bufs=2) as xtp, \
             tc.tile_pool(name="qkp", bufs=2) as qkp, \
             tc.tile_pool(name="vp", bufs=3) as vp, \
             tc.tile_pool(name="otp", bufs=3) as otp, \
             tc.tile_pool(name="work", bufs=6) as work, \
             tc.tile_pool(name="yout", bufs=3) as yout, \
             tc.tile_pool(name="psA", bufs=2, space="PSUM") as psA, \
             tc.tile_pool(name="psT", bufs=2, space="PSUM") as psT:

            # ---- constants ----
            wqk_sb = const.tile([128, 4, 1024], f16)
            nc.gpsimd.dma_start(out=wqk_sb, in_=wqk.rearrange("(k p) m -> p k m", p=128))
            wv_sb = const.tile([128, 4, 512], f16)
            nc.gpsimd.dma_start(out=wv_sb, in_=wv.rearrange("(k p) m -> p k m", p=128))
            wp_sb = const.tile([128, 4, 512], f16)
            nc.gpsimd.dma_start(out=wp_sb, in_=wp.rearrange("(k p) m -> p k m", p=128))
            bias_sb = const.tile([128, 512], f32)
            nc.gpsimd.dma_start(out=bias_sb, in_=bfull[:, :])
            ident_sb = const.tile([128, 128], f16)
            nc.gpsimd.dma_start(out=ident_sb, in_=identd[:, :])
            mask_sb = const.tile([128, NT, 512], f16)
            nc.gpsimd.dma_start(out=mask_sb, in_=maskd.rearrange("t p m -> p t m"))

            for n in range(NB):
                # ---- phase A: projections ----
                xt_sb = xtp.tile([128, 4, 1024], f16)
                nc.gpsimd.dma_start(
                    out=xt_sb, in_=xt[n].rearrange("(k p) s -> p k s", p=128)
                )

                qkT = qkp.tile([128, 8, 1024], f16)
                for m in range(8):
                    ps = psA.tile([128, 1024], f32, tag="A")
                    for k in range(4):
                        for sh in range(2):
                            nc.tensor.matmul(
                                ps[:, sh * 512:(sh + 1) * 512],
                                wqk_sb[:, k, m * 128:(m + 1) * 128],
                                xt_sb[:, k, sh * 512:(sh + 1) * 512],
                                start=(k == 0),
                                stop=(k == 3),
                            )
                    nc.any.tensor_copy(qkT[:, m, :], ps)

                v_ev = vp.tile([128, 8, 512], f16, tag="ve")
                v_od = vp.tile([128, 7, 512], f16, tag="vo")
                for st in range(8):
                    ps = psA.tile([128, 512], f32, tag="A")
                    for k in range(4):
                        nc.tensor.matmul(
                            ps,
                            xt_sb[:, k, st * 128:(st + 1) * 128],
                            wv_sb[:, k, :],
                            start=(k == 0),
                            stop=(k == 3),
                        )
                    nc.any.tensor_copy(v_ev[:, st, :], ps)
                for m in range(7):
                    nc.gpsimd.dma_start(out=v_od[0:64, m, :], in_=v_ev[64:128, m, :])
                    nc.gpsimd.dma_start(out=v_od[64:128, m, :], in_=v_ev[0:64, m + 1, :])

                # ---- phase B: local attention ----
                outT = otp.tile([128, 4, 1024], f16)
                for t in range(NT):
                    stk = _win_start(t)
                    if stk % 128 == 0:
                        vsrc, vbase = v_ev, stk // 128
                    else:
                        vsrc, vbase = v_od, (stk - 64) // 128
                    for hp in range(4):
                        # paired scores: head 2hp on array rows 0:64 ->
                        # bank cols 0:512, head 2hp+1 on rows 64:128 ->
                        # cols 512:1024 (different PSUM banks, concurrent)
                        ps_s = psA.tile([128, 1024], f32, tag="A")
                        for ho in range(2):
                            p0 = ho * 64
                            nc.tensor.matmul(
                                ps_s[:, ho * 512:(ho + 1) * 512],
                                qkT[p0:p0 + 64, hp, t * 128:(t + 1) * 128],
                                qkT[p0:p0 + 64, 4 + hp, stk:stk + 512],
                                start=True,
                                stop=True,
                            )
                        e = work.tile([128, 1024], f16, tag="e")
                        nc.scalar.activation(e, ps_s, mybir.ActivationFunctionType.Exp)

                        ps_o = psT.tile([128, 128], f32, tag="oT")
                        for ho in range(2):
                            h = 2 * hp + ho
                            eM = work.tile([128, 512], f16, tag=f"eM{ho}")
                            z = work.tile([128, 1], f32, tag=f"z{ho}")
                            nc.vector.scalar_tensor_tensor(
                                out=eM, in0=e[:, ho * 512:(ho + 1) * 512],
                                scalar=1.0, in1=mask_sb[:, t, :],
                                op0=mybir.AluOpType.bypass,
                                op1=mybir.AluOpType.mult,
                                accum_out=z,
                            )
                            rz = work.tile([128, 1], f32, tag=f"rz{ho}")
                            nc.vector.reciprocal(rz, z)
                            probs = work.tile([128, 512], f16, tag=f"probs{ho}")
                            nc.vector.tensor_scalar_mul(probs, in0=eM, scalar1=rz)
                            ps_t = psT.tile([128, 512], f16, tag="aT")
                            for j in range(4):
                                nc.tensor.transpose(
                                    ps_t[:, j * 128:(j + 1) * 128],
                                    probs[:, j * 128:(j + 1) * 128],
                                    ident_sb,
                                )
                            aT = work.tile([128, 512], f16, tag=f"aTs{ho}")
                            nc.any.tensor_copy(aT, ps_t)
                            for j in range(4):
                                nc.tensor.matmul(
                                    ps_o[ho * 64:(ho + 1) * 64, :],
                                    vsrc[:, vbase + j, h * 64:(h + 1) * 64],
                                    aT[:, j * 128:(j + 1) * 128],
                                    start=(j == 0),
                                    stop=(j == 3),
                                    skip_group_check=True,
                                )
                        nc.any.tensor_copy(outT[:, hp, t * 128:(t + 1) * 128], ps_o)

                # ---- phase C: output projection ----
                for st in range(8):
                    ps_y = psA.tile([128, 512], f32, tag="A")
                    for k in range(4):
                        nc.tensor.matmul(
                            ps_y,
                            outT[:, k, st * 128:(st + 1) * 128],
                            wp_sb[:, k, :],
                            start=(k == 0),
                            stop=(k == 3),
                        )
                    y_sb = yout.tile([128, 512], f32, tag="y")
                    nc.vector.scalar_tensor_tensor(
                        out=y_sb, in0=ps_y, scalar=1.0, in1=bias_sb,
                        op0=mybir.AluOpType.bypass, op1=mybir.AluOpType.add,
                    )
                    nc.gpsimd.dma_start(
                        out=y[n, st * 128:(st + 1) * 128, :], in_=y_sb
                    )

    nc.finalize()
    return nc


def _install_prof_shim():
    import sys
    import types
    if "antenv.axon_hooks" in sys.modules:
        return
    try:
        from trn_agent_boot.trn_boot import _ntff_profile_via_ctypes
        hook = _ntff_profile_via_ctypes("/opt/axon/libaxon_pjrt.so")
    except Exception:
        return
    mod = types.ModuleType("antenv.axon_hooks")
    mod.get_axon_ntff_profile_hook = lambda: hook
    mod.set_axon_ntff_profile_hook = lambda h: None
    sys.modules["antenv.axon_hooks"] = mod


def _run(inputs, trace=False):
    from concourse.bass_utils import run_bass_kernel_spmd

    if trace:
        _install_prof_shim()
    if "nc" not in _CACHE:
        _CACHE["nc"] = _build_bass()
    nc = _CACHE["nc"]

    x = np.ascontiguousarray(inputs["x"], dtype=np.float32)
    w_qkv = np.asarray(inputs["W_qkv"], dtype=np.float32)
    w_proj = np.ascontiguousarray(inputs["W_proj"], dtype=np.float32)
    b_proj = np.asarray(inputs["b_proj"], dtype=np.float32)

    wqk = w_qkv[:, : 2 * C].copy()
    wqk[:, :C] *= HD ** -0.5
    wqk = wqk.astype(np.float16)
    wv = np.ascontiguousarray(w_qkv[:, 2 * C:]).astype(np.float16)
    wpf = w_proj.astype(np.float16)
    bfull = np.tile(b_proj[None, :], (128, 1)).astype(np.float32)
    ident = np.eye(128, dtype=np.float16)
    masks = _build_masks()

    xtn = x.transpose(0, 2, 1).astype(np.float16)  # [N, C, S]
    in_maps = []
    for c in range(N_CORES):
        in_maps.append({
            "xt": np.ascontiguousarray(xtn[c * NB:(c + 1) * NB]),
            "wqk": wqk, "wv": wv, "wp": wpf,
            "bfull": bfull, "identd": ident, "maskd": masks,
        })

    res = run_bass_kernel_spmd(
        nc, in_maps, core_ids=list(range(N_CORES)), trace=trace
    )
    out = np.concatenate([res.results[c]["y"] for c in range(N_CORES)], axis=0)
    return out.astype(np.float32), res


def kernel(**inputs):
    out, _ = _run(inputs, trace=False)
    return out

